# revision 19
# baseline (speedup 1.0000x reference)
"""Pipelined GEMM kernel for Trainium2, 8 NeuronCores.

Computes C = A @ B + ws*(ws+1)/2 with A:(8192,256) B:(256,8192) fp32.

Sharding: 2x4 grid over (M, N). Core (mi, ni) computes the
(4096, 2048) output block C[mi] x [ni] from A rows [mi] (4MB, staged
K-major since the PE wants the contraction dim on partitions) and B
columns [ni] (2MB). No inter-core communication; per-core HBM traffic is
4 + 2 + 32 = 38MB, vs 41MB for a 1x8 row sharding and vs ~296MB for the
K-parallel + all-reduce layout the hint suggests.

Per-core kernel (Tile framework), memory-bound:
  - A^T shard arrives as 8 x 0.5MB DMAs, B as 4 x 0.5MB DMAs, both cast
    fp32->bf16 (DVE/ACT alternating) in matching 0.5MB pieces so the
    first matmul can start after ~2MB of loads.
  - Main loop over 32 m-tiles: 2(k) x 4(n) bf16 matmuls accumulate into
    [128, 1024] fp32 PSUM tiles (2 banks); +const is fused into the
    PSUM->SBUF copyback (alternating DVE / ACT); two m-tiles share one
    2MB store DMA, alternating between the two HWDGE rings (sync /
    scalar), with the last group split into 0.5MB pieces to shorten the
    kernel's serial tail.
  - bf16 runs the PE at 1 cycle/row (4x the fp32 rate) with fast weight
    loads; bf16 input rounding costs ~1e-3 norm relative error here
    (K=256, N(0,1) data, +36 offset). PSUM accumulation stays fp32.
"""

import contextlib

import numpy as np

import concourse.mybir as mybir
import concourse.tile as tile
from concourse import bacc
from concourse.bass_utils import run_bass_kernel_spmd

M, K, N = 8192, 256, 8192
NCORES = 8
RM, RN = 2, 4  # core grid over (M, N)
MS = M // RM  # 4096 rows of C per core
NS = N // RN  # 2048 cols of C per core
P = 128
MT = MS // P  # 32 m-tiles
KT = K // P  # 2 k-tiles
NCHUNK = 512  # one fp32 PSUM bank / max matmul free dim
NT = NS // NCHUNK  # 4 n-chunks = one [128, 2048] output tile per m-tile
LCHUNK = 1024  # load/cast granularity (0.5MB fp32 per [128, 1024] piece)

F32 = mybir.dt.float32
BF16 = mybir.dt.bfloat16


def build_program(const_add: float, repeat: int = 1, loop_opts: dict | None = None,
                  tail_split: bool = True, stage_bufs: int = 4,
                  opool_bufs: int = 4, psum_bufs: int = 4):
    """repeat>1 wraps the whole body in a HW loop - used only by the
    timing harness (slope between two repeat counts cancels the ~200ms
    axon dispatch overhead)."""
    nc = bacc.Bacc("TRN2", target_bir_lowering=False, debug=False)
    at = nc.dram_tensor("at", [K, MS], F32, kind="ExternalInput")
    b = nc.dram_tensor("b", [K, NS], F32, kind="ExternalInput")
    c = nc.dram_tensor("c", [MS, NS], F32, kind="ExternalOutput")

    with tile.TileContext(nc) as tc:
        with (
            tc.tile_pool(name="stage", bufs=stage_bufs) as stage,
            tc.tile_pool(name="bpool", bufs=1) as bpool,
            tc.tile_pool(name="atpool", bufs=1) as atpool,
            tc.tile_pool(name="psum", bufs=psum_bufs, space="PSUM") as psum_pool,
            tc.tile_pool(name="opool", bufs=opool_bufs) as opool,
            tc.For_i(0, repeat, 1, **(loop_opts or {}))
            if repeat > 1 else contextlib.nullcontext(),
        ):
            at_sb = [
                atpool.tile([P, MS], BF16, name=f"at{k}", tag=f"at{k}")
                for k in range(KT)
            ]
            b_sb = [
                bpool.tile([P, NS], BF16, name=f"b{k}", tag=f"b{k}")
                for k in range(KT)
            ]

            # Interleave the load+cast pieces so what the first m-tiles
            # need arrives first: (at chunk0, b chunk0) then the rest.
            def load_piece(src, dst_bf, col0, width, idx):
                st = stage.tile([P, width], F32, name=f"st{idx}", tag="stage")
                nc.sync.dma_start(st[:], src[:, col0 : col0 + width])
                if idx % 2 == 0:
                    nc.vector.tensor_copy(dst_bf[:, col0 : col0 + width], st[:])
                else:
                    nc.scalar.copy(dst_bf[:, col0 : col0 + width], st[:])

            idx = 0
            for k in range(KT):
                load_piece(at[k * P : (k + 1) * P, :], at_sb[k], 0, LCHUNK, idx)
                idx += 1
            for k in range(KT):
                load_piece(b[k * P : (k + 1) * P, :], b_sb[k], 0, LCHUNK, idx)
                idx += 1
            for k in range(KT):
                load_piece(b[k * P : (k + 1) * P, :], b_sb[k], LCHUNK,
                           NS - LCHUNK, idx)
                idx += 1
            for k in range(KT):
                load_piece(at[k * P : (k + 1) * P, :], at_sb[k], LCHUNK,
                           MS - LCHUNK, idx)
                idx += 1

            # Main GEMM loop; two m-tiles share one output tile so each
            # store DMA moves 2MB.
            for m2 in range(MT // 2):
                ot = opool.tile([P, 2 * NS], F32)
                for mh in range(2):
                    m = m2 * 2 + mh
                    for jj in range(NT // 2):
                        ps = psum_pool.tile([P, 2 * NCHUNK], F32)
                        for j2 in range(2):
                            jc = jj * 2 + j2
                            for k in range(KT):
                                nc.tensor.matmul(
                                    ps[:, j2 * NCHUNK : (j2 + 1) * NCHUNK],
                                    at_sb[k][:, m * P : (m + 1) * P],
                                    b_sb[k][:, jc * NCHUNK : (jc + 1) * NCHUNK],
                                    start=(k == 0),
                                    stop=(k == KT - 1),
                                )
                        # +const fused into PSUM->SBUF eviction
                        dst = ot[:, mh * NS + jj * 2 * NCHUNK
                                 : mh * NS + (jj + 1) * 2 * NCHUNK]
                        if (m + jj) % 2 == 0:
                            nc.vector.tensor_scalar_add(dst, ps[:], const_add)
                        else:
                            nc.scalar.activation(
                                dst, ps[:],
                                mybir.ActivationFunctionType.Copy,
                                bias=const_add,
                            )
                # stores alternate between the two HWDGE rings; the last
                # group is split into 0.5MB pieces on both rings so the
                # kernel's serial tail (final copyback + store drain) is
                # as short as possible.
                if m2 < MT // 2 - 1 or not tail_split:
                    dma_eng = nc.sync if m2 % 2 == 0 else nc.scalar
                    dst_ap = c[m2 * 2 * P : (m2 + 1) * 2 * P, :].rearrange(
                        "(h p) n -> p h n", p=P
                    )
                    dma_eng.dma_start(dst_ap, ot[:])
                else:
                    for mh in range(2):
                        m = m2 * 2 + mh
                        for nh in range(2):
                            dma_eng = nc.sync if nh % 2 == 0 else nc.scalar
                            dma_eng.dma_start(
                                c[m * P : (m + 1) * P,
                                  nh * (NS // 2) : (nh + 1) * (NS // 2)],
                                ot[:, mh * NS + nh * (NS // 2)
                                   : mh * NS + (nh + 1) * (NS // 2)],
                            )

    nc.compile()
    return nc


_CACHE = {}


def _get_program(const_add: float):
    key = const_add
    if key not in _CACHE:
        _CACHE[key] = build_program(const_add)
    return _CACHE[key]


def make_in_maps(A, B):
    """2x4 (M, N) grid; A shards staged K-major."""
    maps = []
    for i in range(NCORES):
        mi, ni = divmod(i, RN)
        maps.append({
            "at": np.ascontiguousarray(A[mi * MS : (mi + 1) * MS].T),
            "b": np.ascontiguousarray(B[:, ni * NS : (ni + 1) * NS]),
        })
    return maps


def assemble(results):
    rows = []
    for mi in range(RM):
        rows.append(np.concatenate(
            [results[mi * RN + ni]["c"] for ni in range(RN)], axis=1))
    return np.concatenate(rows, axis=0)


def run(A, B, world_size, trace=False, **spmd_kwargs):
    A = np.ascontiguousarray(np.asarray(A, dtype=np.float32))
    B = np.ascontiguousarray(np.asarray(B, dtype=np.float32))
    ws = int(world_size)
    const_add = float(ws * (ws + 1) / 2)
    assert A.shape == (M, K) and B.shape == (K, N)

    nc = _get_program(const_add)
    res = run_bass_kernel_spmd(
        nc, make_in_maps(A, B), list(range(NCORES)), trace=trace, **spmd_kwargs
    )
    return assemble(res.results), res


def kernel(A, B, world_size, **_unused):
    out, _ = run(A, B, world_size, trace=False)
    return out
